# revision 33
# baseline (speedup 1.0000x reference)
"""MixTreeLSTMCell Trainium2 kernel (8 NeuronCores, SPMD).

Strategy
--------
The cell evaluates one of two branches per node depending on t in {0,1}.
The host partitions the nodes by type and hands every core an equal
number of type-0 and type-1 nodes (padded up to 512-node tiles), so the
device program has two static segments and no per-node select.

All matmul operands are laid out feature-major on the host (x^T, h^T and
the transposed weight matrices) and cast to fp16 (matmuls accumulate
fp32 in PSUM).  The x/W contraction dim is zero-padded from 301 (300
features + folded-bias ones row) to 384 so every matmul is a uniform
K=128 tile: a K=45 matmul forces a 64-row PE-array reconfig that costs
~+100 ns on itself AND on its successor (measured), so uniform K=128
keeps the whole stream at the 216 ns/matmul roofline.

The iou bias rows are folded into the matmul via the ones row; f-gate
biases are applied per partition by the scalar engine when it drains
PSUM.  The vector engine runs the elementwise chain in fp16 (2x DVE
rate).  Outputs are stored fp16 feature-major and un-permuted/cast on
the host.  Loads are issued in 2048-node macro tiles for 4 KiB-per-
partition DMA runs, split across both HWDGE rings (sync/scalar) with
stores on SWDGE (gpsimd).
"""

from contextlib import ExitStack

import numpy as np

import concourse.bacc as bacc
import concourse.tile as tile
from concourse import mybir
from concourse import bass_utils

F32 = mybir.dt.float32
FP16 = mybir.dt.float16
NP_FP16 = np.float16

N_NODES = 131072
X = 300
XP = X + 1            # x rows + folded-bias ones row
XPAD = 384            # padded to 3 full K=128 tiles (pad rows zero)
H = 256
CORES = 8
TILE_N = 512          # nodes per compute tile (matmul free dim)
MACRO = 4 * TILE_N    # nodes per DMA macro tile

# Set by test harness to profile; LAST_EXEC_NS is filled after each run.
TRACE = False
LAST_EXEC_NS = None

_PROGRAM_CACHE = {}


def _round_up(v, m):
    return (v + m - 1) // m * m


def _tile_plan(P):
    """Split a segment of P columns (multiple of 16) into matmul tiles.

    A sub-128 remainder is folded into the last two tiles (PSUM caps a
    tile at 512, and tiny-N matmuls pay the ~60-cycle NX issue floor 58
    times per tile).  (A trailing 64-col runt tile was tried to shorten
    the tail: it loses more to PE stalls on ACT/DVE turnaround than the
    shorter drain chain saves.)"""
    tiles = []
    rem = P
    while rem > 512:
        if rem < 640:  # would leave a <128 remainder: split evenly
            a = (rem // 2 + 15) // 16 * 16
            tiles += [a, rem - a]
            rem = 0
        else:
            tiles.append(512)
            rem -= 512
    if rem:
        tiles.append(rem)
    return tiles


def _build_program(P0, P1):
    """Trace + compile the SPMD program for P0 type-0 columns and P1
    type-1 columns per core (identical on all cores)."""
    key = (P0, P1)
    if key in _PROGRAM_CACHE:
        return _PROGRAM_CACHE[key]

    Nc = P0 + P1
    nc = bacc.Bacc("TRN2", target_bir_lowering=False, debug=False)

    xT = nc.dram_tensor("xT", [XPAD, Nc], FP16, kind="ExternalInput").ap()
    hT = nc.dram_tensor("hT", [2 * H, Nc], FP16, kind="ExternalInput").ap()
    cT = nc.dram_tensor("cT", [2 * H, Nc], FP16, kind="ExternalInput").ap()

    WnT = nc.dram_tensor("WnT", [XPAD, 3 * H], FP16, kind="ExternalInput").ap()
    UnT = nc.dram_tensor("UnT", [2 * H, 3 * H], FP16, kind="ExternalInput").ap()
    UfwT = nc.dram_tensor("UfwT", [2 * H, 2 * H], FP16, kind="ExternalInput").ap()
    WsT = nc.dram_tensor("WsT", [XPAD, 3 * H], FP16, kind="ExternalInput").ap()
    UsT = nc.dram_tensor("UsT", [H, 3 * H], FP16, kind="ExternalInput").ap()
    UfswT = nc.dram_tensor("UfswT", [H, H], FP16, kind="ExternalInput").ap()

    bias_fn = nc.dram_tensor("bias_fn", [128, 4], F32, kind="ExternalInput").ap()
    bias_fs = nc.dram_tensor("bias_fs", [128, 2], F32, kind="ExternalInput").ap()

    hOT = nc.dram_tensor("hOT", [H, Nc], FP16, kind="ExternalOutput").ap()
    cOT = nc.dram_tensor("cOT", [H, Nc], FP16, kind="ExternalOutput").ap()

    # feature-major [p, ko, n] views of the DRAM activations
    xT_v = xT.rearrange("(ko p) n -> p ko n", p=128)
    hT_v = hT.rearrange("(ko p) n -> p ko n", p=128)
    cT_v = cT.rearrange("(ko p) n -> p ko n", p=128)
    hOT_v = hOT.rearrange("(ko p) n -> p ko n", p=128)
    cOT_v = cOT.rearrange("(ko p) n -> p ko n", p=128)
    WnT_v = WnT.rearrange("(ko p) m -> p ko m", p=128)
    WsT_v = WsT.rearrange("(ko p) m -> p ko m", p=128)
    UnT_v = UnT.rearrange("(ko p) m -> p ko m", p=128)
    UfwT_v = UfwT.rearrange("(ko p) m -> p ko m", p=128)
    UsT_v = UsT.rearrange("(ko p) m -> p ko m", p=128)
    UfswT_v = UfswT.rearrange("(ko p) m -> p ko m", p=128)

    SIG = mybir.ActivationFunctionType.Sigmoid
    TANH = mybir.ActivationFunctionType.Tanh

    with tile.TileContext(nc) as tc, ExitStack() as stack:
        wp = stack.enter_context(tc.tile_pool(name="w", bufs=1))
        io = stack.enter_context(tc.tile_pool(name="io", bufs=3))
        mid = stack.enter_context(tc.tile_pool(name="mid", bufs=2))
        midf = stack.enter_context(tc.tile_pool(name="midf", bufs=3))
        psf = stack.enter_context(tc.tile_pool(name="psf", bufs=4, space="PSUM"))
        ps2 = stack.enter_context(tc.tile_pool(name="ps2", bufs=2, space="PSUM"))

        # --- resident weights.  MM0 (f-gates of tile 0) needs Ufw + ht0:
        # the f-path weights ride the otherwise-idle gpsimd ring so they
        # load in parallel with ht0/xt0 on sync.  Wn/Un (needed ~3.4us in)
        # go on scalar ahead of the first ct; Ws/Us (needed only in the
        # type-1 segment much later) are issued after the second macro. ---
        Ufw_sb = wp.tile([128, 4, 2 * H], FP16)
        # MM0's critical bytes (Ufw + ht0) are balanced over all three
        # rings: Ufw k0-2 gpsimd + k3 scalar, ht0 k01 sync + k23 scalar
        nc.gpsimd.dma_start(out=Ufw_sb[:, 0:3, :], in_=UfwT_v[:, 0:3, :])
        nc.scalar.dma_start(out=Ufw_sb[:, 3, :], in_=UfwT_v[:, 3, :])
        Ufsw_sb = wp.tile([128, 2, H], FP16)
        nc.gpsimd.dma_start(out=Ufsw_sb, in_=UfswT_v)
        bfn_sb = wp.tile([128, 4], F32)
        nc.gpsimd.dma_start(out=bfn_sb, in_=bias_fn)
        bfs_sb = wp.tile([128, 2], F32)
        nc.gpsimd.dma_start(out=bfs_sb, in_=bias_fs)
        Wn_sb = wp.tile([128, 3, 3 * H], FP16)
        Ws_sb = wp.tile([128, 3, 3 * H], FP16)
        Un_sb = wp.tile([128, 4, 3 * H], FP16)
        Us_sb = wp.tile([128, 2, 3 * H], FP16)

        def iou_mm(ps, xt, ht, htild, c0, W_sb, U_sb, uk, m, ncol):
            """All matmuls accumulating iou m-tile m into ps[:, :ncol]."""
            ms = slice(128 * m, 128 * (m + 1))
            ns = slice(c0, c0 + ncol)
            for k in range(3):
                nc.tensor.matmul(
                    ps, W_sb[:, k, ms], xt[:, k, ns], start=(k == 0), stop=False
                )
            for k in range(uk):
                rhs = ht[:, k, ns] if htild is None else htild[:, k, :]
                nc.tensor.matmul(
                    ps, U_sb[:, k, ms], rhs, start=False, stop=(k == uk - 1)
                )

        def do_tile(br, xt, ht, ct, c0, n0, ncol, last=False):
            """Process one <=512-node tile; xt/ht/ct are MACRO tiles, c0 the
            column offset inside the macro, n0 the DRAM node offset."""
            ns = slice(c0, c0 + ncol)

            # --- forget gates f: [128, 4, ncol] = 512 features x nodes ---
            f_full = midf.tile([128, 4, TILE_N], FP16, tag="f", name="f")
            f = f_full[:, :, :ncol]
            if br == 0:
                for m in range(4):
                    ps_full = psf.tile([128, TILE_N], F32, tag="psf", name="ps")
                    ps = ps_full[:, :ncol]
                    for k in range(4):
                        nc.tensor.matmul(
                            ps,
                            Ufw_sb[:, k, 128 * m : 128 * (m + 1)],
                            ht[:, k, ns],
                            start=(k == 0),
                            stop=(k == 3),
                        )
                    nc.scalar.activation(
                        out=f[:, m, :], in_=ps, func=SIG, bias=bfn_sb[:, m : m + 1]
                    )
            else:
                for child in range(2):
                    for m in range(2):
                        ps_full = psf.tile([128, TILE_N], F32, tag="psf", name="ps")
                        ps = ps_full[:, :ncol]
                        for k in range(2):
                            nc.tensor.matmul(
                                ps,
                                Ufsw_sb[:, k, 128 * m : 128 * (m + 1)],
                                ht[:, 2 * child + k, ns],
                                start=(k == 0),
                                stop=(k == 1),
                            )
                        nc.scalar.activation(
                            out=f[:, 2 * child + m, :],
                            in_=ps,
                            func=SIG,
                            bias=bfs_sb[:, m : m + 1],
                        )

            # prod = f * c_child (in place), c_red = child0 + child1
            nc.vector.tensor_mul(out=f, in0=f, in1=ct[:, :, ns])
            cred_full = mid.tile([128, 2, TILE_N], FP16, tag="cred", name="cred")
            cred = cred_full[:, :, :ncol]
            nc.vector.tensor_add(out=cred, in0=f[:, 0:2, :], in1=f[:, 2:4, :])

            htild = None
            if br == 1:
                htild_full = mid.tile([128, 2, TILE_N], FP16, tag="htild", name="htild")
                htild = htild_full[:, :, :ncol]
                nc.vector.tensor_add(out=htild, in0=ht[:, 0:2, ns], in1=ht[:, 2:4, ns])

            # --- iou gates: 3 m-pairs, each a 2-bank PSUM + single ACT ---
            # (iou bias is folded into the matmul via the x^T ones row)
            # m-pair order u, i, o: once u and i are drained, the c chain
            # (mul/add/tanh) is issued BEFORE the o drain in the scalar
            # FIFO, so after the tile's last matmul only ACT(o) + the hout
            # mul + store remain — shortens the per-tile (and program-end)
            # latency tail.  o's matmuls cover the interposed tanh(c).
            gates_full = mid.tile([128, 6, TILE_N], FP16, tag="gates", name="gates")
            gates = gates_full[:, :, :ncol]
            cout_full = mid.tile([128, 2, TILE_N], FP16, tag="cout", name="cout")
            cout = cout_full[:, :, :ncol]
            tct_full = mid.tile([128, 2, TILE_N], FP16, tag="tct", name="tct")
            tct = tct_full[:, :, :ncol]
            for mp in (2, 0, 1):
                ps_full = ps2.tile([128, 2, TILE_N], F32, tag="ps2", name="ps")
                ps = ps_full[:, :, :ncol]
                for m2 in range(2):
                    m = 2 * mp + m2
                    if br == 0:
                        iou_mm(ps[:, m2, :], xt, ht, None, c0, Wn_sb, Un_sb, 4, m, ncol)
                    else:
                        iou_mm(ps[:, m2, :], xt, ht, htild, c0, Ws_sb, Us_sb, 2, m, ncol)
                nc.scalar.activation(
                    out=gates[:, 2 * mp : 2 * mp + 2, :],
                    in_=ps,
                    func=TANH if mp == 2 else SIG,
                )
                if mp == 0:
                    # u and i are drained: c = sig(i)*tanh(u) + c_red
                    nc.vector.tensor_mul(
                        out=cout, in0=gates[:, 0:2, :], in1=gates[:, 4:6, :]
                    )
                    nc.vector.tensor_add(out=cout, in0=cout, in1=cred)
                    nc.scalar.activation(out=tct, in_=cout, func=TANH)

            # h = sig(o)*tanh(c)
            hout_full = mid.tile([128, 2, TILE_N], FP16, tag="hout", name="hout")
            hout = hout_full[:, :, :ncol]
            nc.vector.tensor_mul(out=hout, in0=gates[:, 2:4, :], in1=tct)

            # last tile's stores ride the idle HWDGE rings (h on sync, c on
            # scalar, in parallel) so the program end doesn't wait on a
            # SWDGE drain or a serialized final store
            if last:
                nc.sync.dma_start(out=hOT_v[:, :, n0 : n0 + ncol], in_=hout)
                nc.scalar.dma_start(out=cOT_v[:, :, n0 : n0 + ncol], in_=cout)
            else:
                nc.gpsimd.dma_start(out=hOT_v[:, :, n0 : n0 + ncol], in_=hout)
                nc.gpsimd.dma_start(out=cOT_v[:, :, n0 : n0 + ncol], in_=cout)

        # macro-tile loop: load up to 2048 nodes at a time, then compute the
        # macro's tiles.  The first two macros are single small tiles so the
        # PE ramps up as soon as the first slice lands.  do_tile gets the
        # column offset within the macro plus its width.
        segs = [(0, 0, _tile_plan(P0)), (1, P0, _tile_plan(P1))]
        first = True
        n_macros = 0
        for br, base, tiles in segs:
            macros = []
            i = 0
            while i < len(tiles):
                if first and len(macros) < 2:
                    macros.append([i])
                    i += 1
                    continue
                # the macro right after the two warmup singles is capped at 2
                # tiles so its prefetch doesn't crowd MM0's critical loads
                cap = 2 if (first and len(macros) == 2) else 4
                grp = []
                w = 0
                while i < len(tiles) and len(grp) < cap and w + tiles[i] <= MACRO:
                    grp.append(i)
                    w += tiles[i]
                    i += 1
                macros.append(grp)
            first = False
            off = [0]
            for w_ in tiles:
                off.append(off[-1] + w_)
            for grp in macros:
                n0 = base + off[grp[0]]
                w = off[grp[-1]] + tiles[grp[-1]] - off[grp[0]]
                ht_full = io.tile([128, 4, MACRO], FP16, tag="ht", name="ht")
                ht = ht_full[:, :, :w]
                if n_macros == 0:
                    nc.sync.dma_start(out=ht[:, 0:2, :], in_=hT_v[:, 0:2, n0 : n0 + w])
                    nc.scalar.dma_start(out=ht[:, 2:4, :], in_=hT_v[:, 2:4, n0 : n0 + w])
                else:
                    nc.sync.dma_start(out=ht, in_=hT_v[:, :, n0 : n0 + w])
                xt_full = io.tile([128, 3, MACRO], FP16, tag="xt", name="xt")
                xt = xt_full[:, :, :w]
                nc.sync.dma_start(out=xt, in_=xT_v[:, :, n0 : n0 + w])
                ct_full = io.tile([128, 4, MACRO], FP16, tag="ct", name="ct")
                ct = ct_full[:, :, :w]
                nc.scalar.dma_start(out=ct, in_=cT_v[:, :, n0 : n0 + w])
                n_macros += 1
                # Wn/Un are first needed ~3.5us after MM0, Ws/Us only in the
                # type-1 segment: issue them behind the first macros' loads
                # so MM0's critical Ufw+ht0 own the DMA engines early.
                if n_macros == 1:
                    nc.scalar.dma_start(out=Wn_sb, in_=WnT_v)
                    nc.scalar.dma_start(out=Un_sb, in_=UnT_v)
                elif n_macros == 2:
                    nc.scalar.dma_start(out=Ws_sb, in_=WsT_v)
                    nc.scalar.dma_start(out=Us_sb, in_=UsT_v)
                for ti in grp:
                    do_tile(
                        br, xt, ht, ct,
                        off[ti] - off[grp[0]],
                        base + off[ti],
                        tiles[ti],
                        last=(br == 1 and ti == len(tiles) - 1),
                    )

    nc.compile()
    _PROGRAM_CACHE[key] = nc
    return nc


def kernel(x, h_child, c_child, t, W_iou, U_iou, b_iou, U_f_w, U_f_b,
           W_iou_s, U_iou_s, b_iou_s, U_f_s_w, U_f_s_b):
    global LAST_EXEC_NS
    x = np.asarray(x, dtype=np.float32)
    h_child = np.asarray(h_child, dtype=np.float32)
    c_child = np.asarray(c_child, dtype=np.float32)
    t = np.asarray(t)
    n = x.shape[0]

    # --- host partition: equal per-core type counts, padded to tiles ---
    idx0 = np.flatnonzero(t == 0)
    idx1 = np.flatnonzero(t != 0)
    n0, n1 = len(idx0), len(idx1)

    def pad_split(idx, cnt):
        if cnt == 0:
            return np.zeros((CORES, 0), dtype=np.int64), 0
        per = _round_up(-(-cnt // CORES), 16)
        padded = np.concatenate(
            [idx, np.full(CORES * per - cnt, idx[-1], dtype=idx.dtype)]
        )
        return padded.reshape(CORES, per).astype(np.int64), per

    chunks0, P0 = pad_split(idx0, n0)
    chunks1, P1 = pad_split(idx1, n1)

    nc = _build_program(P0, P1)

    # --- weights (shared across cores) ---
    hc2 = h_child.reshape(n, 2 * H)
    cc2 = c_child.reshape(n, 2 * H)

    def bias_tile(v, m):
        # [m*128] bias vector -> [128, m] per-partition layout
        return np.ascontiguousarray(
            np.asarray(v, np.float32).reshape(-1)[: 128 * m].reshape(m, 128).T
        )

    def w_with_bias(W, b):
        # [XPAD, 768] = W^T with the iou bias as row 300, zero-padded to 384
        out = np.zeros((XPAD, 3 * H), dtype=NP_FP16)
        out[:X] = np.asarray(W, np.float32).T.astype(NP_FP16)
        out[X] = np.asarray(b, np.float32).reshape(-1).astype(NP_FP16)
        return out

    wmap = {
        "WnT": w_with_bias(W_iou, b_iou),
        "UnT": np.ascontiguousarray(np.asarray(U_iou, np.float32).T).astype(NP_FP16),
        "UfwT": np.ascontiguousarray(np.asarray(U_f_w, np.float32).T).astype(NP_FP16),
        "WsT": w_with_bias(W_iou_s, b_iou_s),
        "UsT": np.ascontiguousarray(np.asarray(U_iou_s, np.float32).T).astype(NP_FP16),
        "UfswT": np.ascontiguousarray(np.asarray(U_f_s_w, np.float32).T).astype(NP_FP16),
        "bias_fn": bias_tile(U_f_b, 4),
        "bias_fs": bias_tile(U_f_s_b, 2),
    }

    in_maps = []
    for i in range(CORES):
        I = np.concatenate([chunks0[i], chunks1[i]])
        m = dict(wmap)
        xTi = np.zeros((XPAD, len(I)), dtype=NP_FP16)
        xTi[:X] = x[I].T.astype(NP_FP16)
        xTi[X] = 1.0
        m["xT"] = xTi
        m["hT"] = hc2[I].T.astype(NP_FP16)
        m["cT"] = cc2[I].T.astype(NP_FP16)
        in_maps.append(m)

    res = bass_utils.run_bass_kernel_spmd(
        nc, in_maps, core_ids=list(range(CORES)), trace=TRACE
    )
    LAST_EXEC_NS = res.exec_time_ns

    # --- scatter back ---
    h_out = np.empty((n, H), dtype=np.float32)
    c_out = np.empty((n, H), dtype=np.float32)
    if n0:
        h0 = np.concatenate([res.results[i]["hOT"][:, :P0].T for i in range(CORES)])
        c0 = np.concatenate([res.results[i]["cOT"][:, :P0].T for i in range(CORES)])
        h_out[idx0] = h0[:n0].astype(np.float32)
        c_out[idx0] = c0[:n0].astype(np.float32)
    if n1:
        h1 = np.concatenate([res.results[i]["hOT"][:, P0:].T for i in range(CORES)])
        c1 = np.concatenate([res.results[i]["cOT"][:, P0:].T for i in range(CORES)])
        h_out[idx1] = h1[:n1].astype(np.float32)
        c_out[idx1] = c1[:n1].astype(np.float32)
    return h_out, c_out


# revision 35
# speedup vs baseline: 1.0154x; 1.0154x over previous
"""MixTreeLSTMCell Trainium2 kernel (8 NeuronCores, SPMD).

Strategy
--------
The cell evaluates one of two branches per node depending on t in {0,1}.
The host partitions the nodes by type and hands every core an equal
number of type-0 and type-1 nodes (padded up to 512-node tiles), so the
device program has two static segments and no per-node select.

All matmul operands are laid out feature-major on the host (x^T, h^T and
the transposed weight matrices) and cast to fp16 (matmuls accumulate
fp32 in PSUM).  The x/W contraction dim is zero-padded from 301 (300
features + folded-bias ones row) to 384 so every matmul is a uniform
K=128 tile: a K=45 matmul forces a 64-row PE-array reconfig that costs
~+100 ns on itself AND on its successor (measured), so uniform K=128
keeps the whole stream at the 216 ns/matmul roofline.

The iou bias rows are folded into the matmul via the ones row; f-gate
biases are applied per partition by the scalar engine when it drains
PSUM.  The vector engine runs the elementwise chain in fp16 (2x DVE
rate).  Outputs are stored fp16 feature-major and un-permuted/cast on
the host.  Loads are issued in 2048-node macro tiles for 4 KiB-per-
partition DMA runs, split across both HWDGE rings (sync/scalar) with
stores on SWDGE (gpsimd).
"""

from contextlib import ExitStack

import numpy as np

import concourse.bacc as bacc
import concourse.tile as tile
from concourse import mybir
from concourse import bass_utils

F32 = mybir.dt.float32
FP16 = mybir.dt.float16
NP_FP16 = np.float16

N_NODES = 131072
X = 300
XP = X + 1            # x rows + folded-bias ones row
XPAD = 384            # padded to 3 full K=128 tiles (pad rows zero)
H = 256
CORES = 8
TILE_N = 512          # nodes per compute tile (matmul free dim)
MACRO = 4 * TILE_N    # nodes per DMA macro tile

# Set by test harness to profile; LAST_EXEC_NS is filled after each run.
TRACE = False
LAST_EXEC_NS = None

_PROGRAM_CACHE = {}


def _round_up(v, m):
    return (v + m - 1) // m * m


def _tile_plan(P):
    """Split a segment of P columns (multiple of 16) into matmul tiles.

    A sub-128 remainder is folded into the last two tiles (PSUM caps a
    tile at 512, and tiny-N matmuls pay the ~60-cycle NX issue floor 58
    times per tile).  (A trailing 64-col runt tile was tried to shorten
    the tail: it loses more to PE stalls on ACT/DVE turnaround than the
    shorter drain chain saves.)"""
    tiles = []
    rem = P
    while rem > 512:
        if rem < 640:  # would leave a <128 remainder: split evenly
            a = (rem // 2 + 15) // 16 * 16
            tiles += [a, rem - a]
            rem = 0
        else:
            tiles.append(512)
            rem -= 512
    if rem:
        tiles.append(rem)
    return tiles


def _build_program(P0, P1):
    """Trace + compile the SPMD program for P0 type-0 columns and P1
    type-1 columns per core (identical on all cores)."""
    key = (P0, P1)
    if key in _PROGRAM_CACHE:
        return _PROGRAM_CACHE[key]

    Nc = P0 + P1
    nc = bacc.Bacc("TRN2", target_bir_lowering=False, debug=False)

    xT = nc.dram_tensor("xT", [XPAD, Nc], FP16, kind="ExternalInput").ap()
    hT = nc.dram_tensor("hT", [2 * H, Nc], FP16, kind="ExternalInput").ap()
    cT = nc.dram_tensor("cT", [2 * H, Nc], FP16, kind="ExternalInput").ap()

    WnT = nc.dram_tensor("WnT", [XPAD, 3 * H], FP16, kind="ExternalInput").ap()
    UnT = nc.dram_tensor("UnT", [2 * H, 3 * H], FP16, kind="ExternalInput").ap()
    UfwT = nc.dram_tensor("UfwT", [2 * H, 2 * H], FP16, kind="ExternalInput").ap()
    WsT = nc.dram_tensor("WsT", [XPAD, 3 * H], FP16, kind="ExternalInput").ap()
    UsT = nc.dram_tensor("UsT", [H, 3 * H], FP16, kind="ExternalInput").ap()
    UfswT = nc.dram_tensor("UfswT", [H, H], FP16, kind="ExternalInput").ap()

    bias_fn = nc.dram_tensor("bias_fn", [128, 4], F32, kind="ExternalInput").ap()
    bias_fs = nc.dram_tensor("bias_fs", [128, 2], F32, kind="ExternalInput").ap()

    hOT = nc.dram_tensor("hOT", [H, Nc], FP16, kind="ExternalOutput").ap()
    cOT = nc.dram_tensor("cOT", [H, Nc], FP16, kind="ExternalOutput").ap()

    # feature-major [p, ko, n] views of the DRAM activations
    xT_v = xT.rearrange("(ko p) n -> p ko n", p=128)
    hT_v = hT.rearrange("(ko p) n -> p ko n", p=128)
    cT_v = cT.rearrange("(ko p) n -> p ko n", p=128)
    hOT_v = hOT.rearrange("(ko p) n -> p ko n", p=128)
    cOT_v = cOT.rearrange("(ko p) n -> p ko n", p=128)
    WnT_v = WnT.rearrange("(ko p) m -> p ko m", p=128)
    WsT_v = WsT.rearrange("(ko p) m -> p ko m", p=128)
    UnT_v = UnT.rearrange("(ko p) m -> p ko m", p=128)
    UfwT_v = UfwT.rearrange("(ko p) m -> p ko m", p=128)
    UsT_v = UsT.rearrange("(ko p) m -> p ko m", p=128)
    UfswT_v = UfswT.rearrange("(ko p) m -> p ko m", p=128)

    SIG = mybir.ActivationFunctionType.Sigmoid
    TANH = mybir.ActivationFunctionType.Tanh

    with tile.TileContext(nc) as tc, ExitStack() as stack:
        wp = stack.enter_context(tc.tile_pool(name="w", bufs=1))
        io = stack.enter_context(tc.tile_pool(name="io", bufs=3))
        mid = stack.enter_context(tc.tile_pool(name="mid", bufs=2))
        midf = stack.enter_context(tc.tile_pool(name="midf", bufs=3))
        psf = stack.enter_context(tc.tile_pool(name="psf", bufs=4, space="PSUM"))
        ps2 = stack.enter_context(tc.tile_pool(name="ps2", bufs=2, space="PSUM"))

        # --- resident weights.  MM0 (f-gates of tile 0) needs Ufw + ht0:
        # the f-path weights ride the otherwise-idle gpsimd ring so they
        # load in parallel with ht0/xt0 on sync.  Wn/Un (needed ~3.4us in)
        # go on scalar ahead of the first ct; Ws/Us (needed only in the
        # type-1 segment much later) are issued after the second macro. ---
        Ufw_sb = wp.tile([128, 4, 2 * H], FP16)
        # (Splitting Ufw/ht0 across the scalar ring moves MM0 ~1.2us
        # earlier but starves tiles 1-2 behind the delayed ct0/Wn —
        # net loss, so the critical pair stays on gpsimd+sync.)
        nc.gpsimd.dma_start(out=Ufw_sb, in_=UfwT_v)
        Ufsw_sb = wp.tile([128, 2, H], FP16)
        nc.gpsimd.dma_start(out=Ufsw_sb, in_=UfswT_v)
        bfn_sb = wp.tile([128, 4], F32)
        nc.gpsimd.dma_start(out=bfn_sb, in_=bias_fn)
        bfs_sb = wp.tile([128, 2], F32)
        nc.gpsimd.dma_start(out=bfs_sb, in_=bias_fs)
        Wn_sb = wp.tile([128, 3, 3 * H], FP16)
        Ws_sb = wp.tile([128, 3, 3 * H], FP16)
        Un_sb = wp.tile([128, 4, 3 * H], FP16)
        Us_sb = wp.tile([128, 2, 3 * H], FP16)

        def iou_mm(ps, xt, ht, htild, c0, W_sb, U_sb, uk, m, ncol):
            """All matmuls accumulating iou m-tile m into ps[:, :ncol]."""
            ms = slice(128 * m, 128 * (m + 1))
            ns = slice(c0, c0 + ncol)
            for k in range(3):
                nc.tensor.matmul(
                    ps, W_sb[:, k, ms], xt[:, k, ns], start=(k == 0), stop=False
                )
            for k in range(uk):
                rhs = ht[:, k, ns] if htild is None else htild[:, k, :]
                nc.tensor.matmul(
                    ps, U_sb[:, k, ms], rhs, start=False, stop=(k == uk - 1)
                )

        def do_tile(br, xt, ht, ct, c0, n0, ncol, last=False):
            """Process one <=512-node tile; xt/ht/ct are MACRO tiles, c0 the
            column offset inside the macro, n0 the DRAM node offset."""
            ns = slice(c0, c0 + ncol)

            # --- forget gates f: [128, 4, ncol] = 512 features x nodes ---
            f_full = midf.tile([128, 4, TILE_N], FP16, tag="f", name="f")
            f = f_full[:, :, :ncol]
            if br == 0:
                for m in range(4):
                    ps_full = psf.tile([128, TILE_N], F32, tag="psf", name="ps")
                    ps = ps_full[:, :ncol]
                    for k in range(4):
                        nc.tensor.matmul(
                            ps,
                            Ufw_sb[:, k, 128 * m : 128 * (m + 1)],
                            ht[:, k, ns],
                            start=(k == 0),
                            stop=(k == 3),
                        )
                    nc.scalar.activation(
                        out=f[:, m, :], in_=ps, func=SIG, bias=bfn_sb[:, m : m + 1]
                    )
            else:
                for child in range(2):
                    for m in range(2):
                        ps_full = psf.tile([128, TILE_N], F32, tag="psf", name="ps")
                        ps = ps_full[:, :ncol]
                        for k in range(2):
                            nc.tensor.matmul(
                                ps,
                                Ufsw_sb[:, k, 128 * m : 128 * (m + 1)],
                                ht[:, 2 * child + k, ns],
                                start=(k == 0),
                                stop=(k == 1),
                            )
                        nc.scalar.activation(
                            out=f[:, 2 * child + m, :],
                            in_=ps,
                            func=SIG,
                            bias=bfs_sb[:, m : m + 1],
                        )

            # prod = f * c_child (in place), c_red = child0 + child1
            nc.vector.tensor_mul(out=f, in0=f, in1=ct[:, :, ns])
            cred_full = mid.tile([128, 2, TILE_N], FP16, tag="cred", name="cred")
            cred = cred_full[:, :, :ncol]
            nc.vector.tensor_add(out=cred, in0=f[:, 0:2, :], in1=f[:, 2:4, :])

            htild = None
            if br == 1:
                htild_full = mid.tile([128, 2, TILE_N], FP16, tag="htild", name="htild")
                htild = htild_full[:, :, :ncol]
                nc.vector.tensor_add(out=htild, in0=ht[:, 0:2, ns], in1=ht[:, 2:4, ns])

            # --- iou gates: 3 m-pairs, each a 2-bank PSUM + single ACT ---
            # (iou bias is folded into the matmul via the x^T ones row)
            # m-pair order u, i, o: once u and i are drained, the c chain
            # (mul/add/tanh) is issued BEFORE the o drain in the scalar
            # FIFO, so after the tile's last matmul only ACT(o) + the hout
            # mul + store remain — shortens the per-tile (and program-end)
            # latency tail.  o's matmuls cover the interposed tanh(c).
            gates_full = mid.tile([128, 6, TILE_N], FP16, tag="gates", name="gates")
            gates = gates_full[:, :, :ncol]
            cout_full = mid.tile([128, 2, TILE_N], FP16, tag="cout", name="cout")
            cout = cout_full[:, :, :ncol]
            tct_full = mid.tile([128, 2, TILE_N], FP16, tag="tct", name="tct")
            tct = tct_full[:, :, :ncol]
            for mp in (2, 0, 1):
                ps_full = ps2.tile([128, 2, TILE_N], F32, tag="ps2", name="ps")
                ps = ps_full[:, :, :ncol]
                for m2 in range(2):
                    m = 2 * mp + m2
                    if br == 0:
                        iou_mm(ps[:, m2, :], xt, ht, None, c0, Wn_sb, Un_sb, 4, m, ncol)
                    else:
                        iou_mm(ps[:, m2, :], xt, ht, htild, c0, Ws_sb, Us_sb, 2, m, ncol)
                nc.scalar.activation(
                    out=gates[:, 2 * mp : 2 * mp + 2, :],
                    in_=ps,
                    func=TANH if mp == 2 else SIG,
                )
                if mp == 0:
                    # u and i are drained: c = sig(i)*tanh(u) + c_red
                    nc.vector.tensor_mul(
                        out=cout, in0=gates[:, 0:2, :], in1=gates[:, 4:6, :]
                    )
                    nc.vector.tensor_add(out=cout, in0=cout, in1=cred)
                    nc.scalar.activation(out=tct, in_=cout, func=TANH)

            # h = sig(o)*tanh(c)
            hout_full = mid.tile([128, 2, TILE_N], FP16, tag="hout", name="hout")
            hout = hout_full[:, :, :ncol]
            nc.vector.tensor_mul(out=hout, in0=gates[:, 2:4, :], in1=tct)

            # last tile's stores ride the idle HWDGE rings (h on sync, c on
            # scalar, in parallel) so the program end doesn't wait on a
            # SWDGE drain or a serialized final store
            if last:
                nc.sync.dma_start(out=hOT_v[:, :, n0 : n0 + ncol], in_=hout)
                nc.scalar.dma_start(out=cOT_v[:, :, n0 : n0 + ncol], in_=cout)
            else:
                nc.gpsimd.dma_start(out=hOT_v[:, :, n0 : n0 + ncol], in_=hout)
                nc.gpsimd.dma_start(out=cOT_v[:, :, n0 : n0 + ncol], in_=cout)

        # macro-tile loop: load up to 2048 nodes at a time, then compute the
        # macro's tiles.  The first two macros are single small tiles so the
        # PE ramps up as soon as the first slice lands.  do_tile gets the
        # column offset within the macro plus its width.
        segs = [(0, 0, _tile_plan(P0)), (1, P0, _tile_plan(P1))]
        first = True
        n_macros = 0
        for br, base, tiles in segs:
            macros = []
            i = 0
            while i < len(tiles):
                if first and len(macros) < 2:
                    macros.append([i])
                    i += 1
                    continue
                # the macro right after the two warmup singles is capped at 2
                # tiles so its prefetch doesn't crowd MM0's critical loads
                cap = 2 if (first and len(macros) == 2) else 4
                grp = []
                w = 0
                while i < len(tiles) and len(grp) < cap and w + tiles[i] <= MACRO:
                    grp.append(i)
                    w += tiles[i]
                    i += 1
                macros.append(grp)
            first = False
            off = [0]
            for w_ in tiles:
                off.append(off[-1] + w_)
            for grp in macros:
                n0 = base + off[grp[0]]
                w = off[grp[-1]] + tiles[grp[-1]] - off[grp[0]]
                ht_full = io.tile([128, 4, MACRO], FP16, tag="ht", name="ht")
                ht = ht_full[:, :, :w]
                nc.sync.dma_start(out=ht, in_=hT_v[:, :, n0 : n0 + w])
                xt_full = io.tile([128, 3, MACRO], FP16, tag="xt", name="xt")
                xt = xt_full[:, :, :w]
                nc.sync.dma_start(out=xt, in_=xT_v[:, :, n0 : n0 + w])
                ct_full = io.tile([128, 4, MACRO], FP16, tag="ct", name="ct")
                ct = ct_full[:, :, :w]
                nc.scalar.dma_start(out=ct, in_=cT_v[:, :, n0 : n0 + w])
                n_macros += 1
                # Wn/Un are first needed ~3.5us after MM0, Ws/Us only in the
                # type-1 segment: issue them behind the first macros' loads
                # so MM0's critical Ufw+ht0 own the DMA engines early.
                if n_macros == 1:
                    nc.scalar.dma_start(out=Wn_sb, in_=WnT_v)
                    nc.scalar.dma_start(out=Un_sb, in_=UnT_v)
                elif n_macros == 2:
                    nc.scalar.dma_start(out=Ws_sb, in_=WsT_v)
                    nc.scalar.dma_start(out=Us_sb, in_=UsT_v)
                for ti in grp:
                    do_tile(
                        br, xt, ht, ct,
                        off[ti] - off[grp[0]],
                        base + off[ti],
                        tiles[ti],
                        last=(br == 1 and ti == len(tiles) - 1),
                    )

    nc.compile()
    _PROGRAM_CACHE[key] = nc
    return nc


def kernel(x, h_child, c_child, t, W_iou, U_iou, b_iou, U_f_w, U_f_b,
           W_iou_s, U_iou_s, b_iou_s, U_f_s_w, U_f_s_b):
    global LAST_EXEC_NS
    x = np.asarray(x, dtype=np.float32)
    h_child = np.asarray(h_child, dtype=np.float32)
    c_child = np.asarray(c_child, dtype=np.float32)
    t = np.asarray(t)
    n = x.shape[0]

    # --- host partition: equal per-core type counts, padded to tiles ---
    idx0 = np.flatnonzero(t == 0)
    idx1 = np.flatnonzero(t != 0)
    n0, n1 = len(idx0), len(idx1)

    def pad_split(idx, cnt):
        if cnt == 0:
            return np.zeros((CORES, 0), dtype=np.int64), 0
        per = _round_up(-(-cnt // CORES), 16)
        padded = np.concatenate(
            [idx, np.full(CORES * per - cnt, idx[-1], dtype=idx.dtype)]
        )
        return padded.reshape(CORES, per).astype(np.int64), per

    chunks0, P0 = pad_split(idx0, n0)
    chunks1, P1 = pad_split(idx1, n1)

    nc = _build_program(P0, P1)

    # --- weights (shared across cores) ---
    hc2 = h_child.reshape(n, 2 * H)
    cc2 = c_child.reshape(n, 2 * H)

    def bias_tile(v, m):
        # [m*128] bias vector -> [128, m] per-partition layout
        return np.ascontiguousarray(
            np.asarray(v, np.float32).reshape(-1)[: 128 * m].reshape(m, 128).T
        )

    def w_with_bias(W, b):
        # [XPAD, 768] = W^T with the iou bias as row 300, zero-padded to 384
        out = np.zeros((XPAD, 3 * H), dtype=NP_FP16)
        out[:X] = np.asarray(W, np.float32).T.astype(NP_FP16)
        out[X] = np.asarray(b, np.float32).reshape(-1).astype(NP_FP16)
        return out

    wmap = {
        "WnT": w_with_bias(W_iou, b_iou),
        "UnT": np.ascontiguousarray(np.asarray(U_iou, np.float32).T).astype(NP_FP16),
        "UfwT": np.ascontiguousarray(np.asarray(U_f_w, np.float32).T).astype(NP_FP16),
        "WsT": w_with_bias(W_iou_s, b_iou_s),
        "UsT": np.ascontiguousarray(np.asarray(U_iou_s, np.float32).T).astype(NP_FP16),
        "UfswT": np.ascontiguousarray(np.asarray(U_f_s_w, np.float32).T).astype(NP_FP16),
        "bias_fn": bias_tile(U_f_b, 4),
        "bias_fs": bias_tile(U_f_s_b, 2),
    }

    in_maps = []
    for i in range(CORES):
        I = np.concatenate([chunks0[i], chunks1[i]])
        m = dict(wmap)
        xTi = np.zeros((XPAD, len(I)), dtype=NP_FP16)
        xTi[:X] = x[I].T.astype(NP_FP16)
        xTi[X] = 1.0
        m["xT"] = xTi
        m["hT"] = hc2[I].T.astype(NP_FP16)
        m["cT"] = cc2[I].T.astype(NP_FP16)
        in_maps.append(m)

    res = bass_utils.run_bass_kernel_spmd(
        nc, in_maps, core_ids=list(range(CORES)), trace=TRACE
    )
    LAST_EXEC_NS = res.exec_time_ns

    # --- scatter back ---
    h_out = np.empty((n, H), dtype=np.float32)
    c_out = np.empty((n, H), dtype=np.float32)
    if n0:
        h0 = np.concatenate([res.results[i]["hOT"][:, :P0].T for i in range(CORES)])
        c0 = np.concatenate([res.results[i]["cOT"][:, :P0].T for i in range(CORES)])
        h_out[idx0] = h0[:n0].astype(np.float32)
        c_out[idx0] = c0[:n0].astype(np.float32)
    if n1:
        h1 = np.concatenate([res.results[i]["hOT"][:, P0:].T for i in range(CORES)])
        c1 = np.concatenate([res.results[i]["cOT"][:, P0:].T for i in range(CORES)])
        h_out[idx1] = h1[:n1].astype(np.float32)
        c_out[idx1] = c1[:n1].astype(np.float32)
    return h_out, c_out


# revision 37
# speedup vs baseline: 1.0215x; 1.0060x over previous
"""MixTreeLSTMCell Trainium2 kernel (8 NeuronCores, SPMD).

Strategy
--------
The cell evaluates one of two branches per node depending on t in {0,1}.
The host partitions the nodes by type and hands every core an equal
number of type-0 and type-1 nodes (padded up to 512-node tiles), so the
device program has two static segments and no per-node select.

All matmul operands are laid out feature-major on the host (x^T, h^T and
the transposed weight matrices) and cast to fp16 (matmuls accumulate
fp32 in PSUM).  The x/W contraction dim is zero-padded from 301 (300
features + folded-bias ones row) to 384 so every matmul is a uniform
K=128 tile: a K=45 matmul forces a 64-row PE-array reconfig that costs
~+100 ns on itself AND on its successor (measured), so uniform K=128
keeps the whole stream at the 216 ns/matmul roofline.

The iou bias rows are folded into the matmul via the ones row; f-gate
biases are applied per partition by the scalar engine when it drains
PSUM.  The vector engine runs the elementwise chain in fp16 (2x DVE
rate).  Outputs are stored fp16 feature-major and un-permuted/cast on
the host.  Loads are issued in 2048-node macro tiles for 4 KiB-per-
partition DMA runs, split across both HWDGE rings (sync/scalar) with
stores on SWDGE (gpsimd).
"""

from contextlib import ExitStack

import numpy as np

import concourse.bacc as bacc
import concourse.tile as tile
from concourse import mybir
from concourse import bass_utils

F32 = mybir.dt.float32
FP16 = mybir.dt.float16
NP_FP16 = np.float16

N_NODES = 131072
X = 300
XP = X + 1            # x rows + folded-bias ones row
XPAD = 384            # padded to 3 full K=128 tiles (pad rows zero)
H = 256
CORES = 8
TILE_N = 512          # nodes per compute tile (matmul free dim)
MACRO = 4 * TILE_N    # nodes per DMA macro tile

# Set by test harness to profile; LAST_EXEC_NS is filled after each run.
TRACE = False
LAST_EXEC_NS = None

_PROGRAM_CACHE = {}


def _round_up(v, m):
    return (v + m - 1) // m * m


def _tile_plan(P):
    """Split a segment of P columns (multiple of 16) into matmul tiles.

    A sub-128 remainder is folded into the last two tiles (PSUM caps a
    tile at 512, and tiny-N matmuls pay the ~60-cycle NX issue floor 58
    times per tile).  (A trailing 64-col runt tile was tried to shorten
    the tail: it loses more to PE stalls on ACT/DVE turnaround than the
    shorter drain chain saves.)"""
    tiles = []
    rem = P
    while rem > 512:
        if rem < 640:  # would leave a <128 remainder: split evenly
            a = (rem // 2 + 15) // 16 * 16
            tiles += [a, rem - a]
            rem = 0
        else:
            tiles.append(512)
            rem -= 512
    if rem:
        tiles.append(rem)
    return tiles


def _build_program(P0, P1):
    """Trace + compile the SPMD program for P0 type-0 columns and P1
    type-1 columns per core (identical on all cores)."""
    key = (P0, P1)
    if key in _PROGRAM_CACHE:
        return _PROGRAM_CACHE[key]

    Nc = P0 + P1
    nc = bacc.Bacc("TRN2", target_bir_lowering=False, debug=False)

    xT = nc.dram_tensor("xT", [XPAD, Nc], FP16, kind="ExternalInput").ap()
    hT = nc.dram_tensor("hT", [2 * H, Nc], FP16, kind="ExternalInput").ap()
    cT = nc.dram_tensor("cT", [2 * H, Nc], FP16, kind="ExternalInput").ap()

    WnT = nc.dram_tensor("WnT", [XPAD, 3 * H], FP16, kind="ExternalInput").ap()
    UnT = nc.dram_tensor("UnT", [2 * H, 3 * H], FP16, kind="ExternalInput").ap()
    UfwT = nc.dram_tensor("UfwT", [2 * H, 2 * H], FP16, kind="ExternalInput").ap()
    WsT = nc.dram_tensor("WsT", [XPAD, 3 * H], FP16, kind="ExternalInput").ap()
    UsT = nc.dram_tensor("UsT", [H, 3 * H], FP16, kind="ExternalInput").ap()
    UfswT = nc.dram_tensor("UfswT", [H, H], FP16, kind="ExternalInput").ap()

    bias_fn = nc.dram_tensor("bias_fn", [128, 4], F32, kind="ExternalInput").ap()
    bias_fs = nc.dram_tensor("bias_fs", [128, 2], F32, kind="ExternalInput").ap()

    hOT = nc.dram_tensor("hOT", [H, Nc], FP16, kind="ExternalOutput").ap()
    cOT = nc.dram_tensor("cOT", [H, Nc], FP16, kind="ExternalOutput").ap()

    # feature-major [p, ko, n] views of the DRAM activations
    xT_v = xT.rearrange("(ko p) n -> p ko n", p=128)
    hT_v = hT.rearrange("(ko p) n -> p ko n", p=128)
    cT_v = cT.rearrange("(ko p) n -> p ko n", p=128)
    hOT_v = hOT.rearrange("(ko p) n -> p ko n", p=128)
    cOT_v = cOT.rearrange("(ko p) n -> p ko n", p=128)
    WnT_v = WnT.rearrange("(ko p) m -> p ko m", p=128)
    WsT_v = WsT.rearrange("(ko p) m -> p ko m", p=128)
    UnT_v = UnT.rearrange("(ko p) m -> p ko m", p=128)
    UfwT_v = UfwT.rearrange("(ko p) m -> p ko m", p=128)
    UsT_v = UsT.rearrange("(ko p) m -> p ko m", p=128)
    UfswT_v = UfswT.rearrange("(ko p) m -> p ko m", p=128)

    SIG = mybir.ActivationFunctionType.Sigmoid
    TANH = mybir.ActivationFunctionType.Tanh

    with tile.TileContext(nc) as tc, ExitStack() as stack:
        wp = stack.enter_context(tc.tile_pool(name="w", bufs=1))
        io = stack.enter_context(tc.tile_pool(name="io", bufs=3))
        mid = stack.enter_context(tc.tile_pool(name="mid", bufs=2))
        midf = stack.enter_context(tc.tile_pool(name="midf", bufs=3))
        psf = stack.enter_context(tc.tile_pool(name="psf", bufs=4, space="PSUM"))
        ps2 = stack.enter_context(tc.tile_pool(name="ps2", bufs=2, space="PSUM"))

        # --- resident weights.  MM0 (f-gates of tile 0) needs Ufw + ht0:
        # the f-path weights ride the otherwise-idle gpsimd ring so they
        # load in parallel with ht0/xt0 on sync.  Wn/Un (needed ~3.4us in)
        # go on scalar ahead of the first ct; Ws/Us (needed only in the
        # type-1 segment much later) are issued after the second macro. ---
        Ufw_sb = wp.tile([128, 4, 2 * H], FP16)
        # (Splitting Ufw/ht0 across the scalar ring moves MM0 ~1.2us
        # earlier but starves tiles 1-2 behind the delayed ct0/Wn —
        # net loss, so the critical pair stays on gpsimd+sync.)
        nc.gpsimd.dma_start(out=Ufw_sb, in_=UfwT_v)
        Ufsw_sb = wp.tile([128, 2, H], FP16)
        nc.gpsimd.dma_start(out=Ufsw_sb, in_=UfswT_v)
        bfn_sb = wp.tile([128, 4], F32)
        nc.gpsimd.dma_start(out=bfn_sb, in_=bias_fn)
        bfs_sb = wp.tile([128, 2], F32)
        nc.gpsimd.dma_start(out=bfs_sb, in_=bias_fs)
        Wn_sb = wp.tile([128, 3, 3 * H], FP16)
        Ws_sb = wp.tile([128, 3, 3 * H], FP16)
        Un_sb = wp.tile([128, 4, 3 * H], FP16)
        Us_sb = wp.tile([128, 2, 3 * H], FP16)

        def iou_mm(ps, xt, ht, htild, c0, W_sb, U_sb, uk, m, ncol):
            """All matmuls accumulating iou m-tile m into ps[:, :ncol]."""
            ms = slice(128 * m, 128 * (m + 1))
            ns = slice(c0, c0 + ncol)
            for k in range(3):
                nc.tensor.matmul(
                    ps, W_sb[:, k, ms], xt[:, k, ns], start=(k == 0), stop=False
                )
            for k in range(uk):
                rhs = ht[:, k, ns] if htild is None else htild[:, k, :]
                nc.tensor.matmul(
                    ps, U_sb[:, k, ms], rhs, start=False, stop=(k == uk - 1)
                )

        def do_tile(br, xt, ht, ct, c0, n0, ncol, last=False):
            """Process one <=512-node tile; xt/ht/ct are MACRO tiles, c0 the
            column offset inside the macro, n0 the DRAM node offset."""
            ns = slice(c0, c0 + ncol)

            # --- forget gates f: [128, 4, ncol] = 512 features x nodes ---
            f_full = midf.tile([128, 4, TILE_N], FP16, tag="f", name="f")
            f = f_full[:, :, :ncol]
            if br == 0:
                for m in range(4):
                    ps_full = psf.tile([128, TILE_N], F32, tag="psf", name="ps")
                    ps = ps_full[:, :ncol]
                    for k in range(4):
                        nc.tensor.matmul(
                            ps,
                            Ufw_sb[:, k, 128 * m : 128 * (m + 1)],
                            ht[:, k, ns],
                            start=(k == 0),
                            stop=(k == 3),
                        )
                    nc.scalar.activation(
                        out=f[:, m, :], in_=ps, func=SIG, bias=bfn_sb[:, m : m + 1]
                    )
            else:
                for child in range(2):
                    for m in range(2):
                        ps_full = psf.tile([128, TILE_N], F32, tag="psf", name="ps")
                        ps = ps_full[:, :ncol]
                        for k in range(2):
                            nc.tensor.matmul(
                                ps,
                                Ufsw_sb[:, k, 128 * m : 128 * (m + 1)],
                                ht[:, 2 * child + k, ns],
                                start=(k == 0),
                                stop=(k == 1),
                            )
                        nc.scalar.activation(
                            out=f[:, 2 * child + m, :],
                            in_=ps,
                            func=SIG,
                            bias=bfs_sb[:, m : m + 1],
                        )

            # prod = f * c_child (in place), c_red = child0 + child1
            nc.vector.tensor_mul(out=f, in0=f, in1=ct[:, :, ns])
            cred_full = mid.tile([128, 2, TILE_N], FP16, tag="cred", name="cred")
            cred = cred_full[:, :, :ncol]
            nc.vector.tensor_add(out=cred, in0=f[:, 0:2, :], in1=f[:, 2:4, :])

            htild = None
            if br == 1:
                htild_full = mid.tile([128, 2, TILE_N], FP16, tag="htild", name="htild")
                htild = htild_full[:, :, :ncol]
                nc.vector.tensor_add(out=htild, in0=ht[:, 0:2, ns], in1=ht[:, 2:4, ns])

            # --- iou gates: 3 m-pairs, each a 2-bank PSUM + single ACT ---
            # (iou bias is folded into the matmul via the x^T ones row)
            # m-pair order u, i, o.  On the LAST tile only, the c chain
            # (mul/add/tanh) is issued BEFORE the o drain in the scalar
            # FIFO, so after the program's last matmul only ACT(o) + the
            # hout mul + store remain (~1us tail instead of ~2.2us).  On
            # all other tiles the chain goes AFTER the o drain: the
            # interposed tanh depends on the vector engine and can block
            # the scalar FIFO (observed as ~1.3us PE stalls early in the
            # stream when the DVE lags).
            gates_full = mid.tile([128, 6, TILE_N], FP16, tag="gates", name="gates")
            gates = gates_full[:, :, :ncol]
            cout_full = mid.tile([128, 2, TILE_N], FP16, tag="cout", name="cout")
            cout = cout_full[:, :, :ncol]
            tct_full = mid.tile([128, 2, TILE_N], FP16, tag="tct", name="tct")
            tct = tct_full[:, :, :ncol]
            for mp in (2, 0, 1):
                ps_full = ps2.tile([128, 2, TILE_N], F32, tag="ps2", name="ps")
                ps = ps_full[:, :, :ncol]
                for m2 in range(2):
                    m = 2 * mp + m2
                    if br == 0:
                        iou_mm(ps[:, m2, :], xt, ht, None, c0, Wn_sb, Un_sb, 4, m, ncol)
                    else:
                        iou_mm(ps[:, m2, :], xt, ht, htild, c0, Ws_sb, Us_sb, 2, m, ncol)
                nc.scalar.activation(
                    out=gates[:, 2 * mp : 2 * mp + 2, :],
                    in_=ps,
                    func=TANH if mp == 2 else SIG,
                )
                if mp == 0 and last:
                    # u and i are drained: c = sig(i)*tanh(u) + c_red
                    nc.vector.tensor_mul(
                        out=cout, in0=gates[:, 0:2, :], in1=gates[:, 4:6, :]
                    )
                    nc.vector.tensor_add(out=cout, in0=cout, in1=cred)
                    nc.scalar.activation(out=tct, in_=cout, func=TANH)
            if not last:
                nc.vector.tensor_mul(
                    out=cout, in0=gates[:, 0:2, :], in1=gates[:, 4:6, :]
                )
                nc.vector.tensor_add(out=cout, in0=cout, in1=cred)
                nc.scalar.activation(out=tct, in_=cout, func=TANH)

            # h = sig(o)*tanh(c)
            hout_full = mid.tile([128, 2, TILE_N], FP16, tag="hout", name="hout")
            hout = hout_full[:, :, :ncol]
            nc.vector.tensor_mul(out=hout, in0=gates[:, 2:4, :], in1=tct)

            # last tile's stores ride the idle HWDGE rings (h on sync, c on
            # scalar, in parallel) so the program end doesn't wait on a
            # SWDGE drain or a serialized final store
            if last:
                nc.sync.dma_start(out=hOT_v[:, :, n0 : n0 + ncol], in_=hout)
                nc.scalar.dma_start(out=cOT_v[:, :, n0 : n0 + ncol], in_=cout)
            else:
                nc.gpsimd.dma_start(out=hOT_v[:, :, n0 : n0 + ncol], in_=hout)
                nc.gpsimd.dma_start(out=cOT_v[:, :, n0 : n0 + ncol], in_=cout)

        # macro-tile loop: load up to 2048 nodes at a time, then compute the
        # macro's tiles.  The first two macros are single small tiles so the
        # PE ramps up as soon as the first slice lands.  do_tile gets the
        # column offset within the macro plus its width.
        segs = [(0, 0, _tile_plan(P0)), (1, P0, _tile_plan(P1))]
        first = True
        n_macros = 0
        for br, base, tiles in segs:
            macros = []
            i = 0
            while i < len(tiles):
                if first and len(macros) < 2:
                    macros.append([i])
                    i += 1
                    continue
                # the macro right after the two warmup singles is capped at 2
                # tiles so its prefetch doesn't crowd MM0's critical loads
                cap = 2 if (first and len(macros) == 2) else 4
                grp = []
                w = 0
                while i < len(tiles) and len(grp) < cap and w + tiles[i] <= MACRO:
                    grp.append(i)
                    w += tiles[i]
                    i += 1
                macros.append(grp)
            first = False
            off = [0]
            for w_ in tiles:
                off.append(off[-1] + w_)
            for grp in macros:
                n0 = base + off[grp[0]]
                w = off[grp[-1]] + tiles[grp[-1]] - off[grp[0]]
                ht_full = io.tile([128, 4, MACRO], FP16, tag="ht", name="ht")
                ht = ht_full[:, :, :w]
                nc.sync.dma_start(out=ht, in_=hT_v[:, :, n0 : n0 + w])
                xt_full = io.tile([128, 3, MACRO], FP16, tag="xt", name="xt")
                xt = xt_full[:, :, :w]
                nc.sync.dma_start(out=xt, in_=xT_v[:, :, n0 : n0 + w])
                ct_full = io.tile([128, 4, MACRO], FP16, tag="ct", name="ct")
                ct = ct_full[:, :, :w]
                nc.scalar.dma_start(out=ct, in_=cT_v[:, :, n0 : n0 + w])
                n_macros += 1
                # Wn/Un are first needed ~3.5us after MM0, Ws/Us only in the
                # type-1 segment: issue them behind the first macros' loads
                # so MM0's critical Ufw+ht0 own the DMA engines early.
                if n_macros == 1:
                    nc.scalar.dma_start(out=Wn_sb, in_=WnT_v)
                    nc.scalar.dma_start(out=Un_sb, in_=UnT_v)
                elif n_macros == 2:
                    nc.scalar.dma_start(out=Ws_sb, in_=WsT_v)
                    nc.scalar.dma_start(out=Us_sb, in_=UsT_v)
                for ti in grp:
                    do_tile(
                        br, xt, ht, ct,
                        off[ti] - off[grp[0]],
                        base + off[ti],
                        tiles[ti],
                        last=(br == 1 and ti == len(tiles) - 1),
                    )

    nc.compile()
    _PROGRAM_CACHE[key] = nc
    return nc


def kernel(x, h_child, c_child, t, W_iou, U_iou, b_iou, U_f_w, U_f_b,
           W_iou_s, U_iou_s, b_iou_s, U_f_s_w, U_f_s_b):
    global LAST_EXEC_NS
    x = np.asarray(x, dtype=np.float32)
    h_child = np.asarray(h_child, dtype=np.float32)
    c_child = np.asarray(c_child, dtype=np.float32)
    t = np.asarray(t)
    n = x.shape[0]

    # --- host partition: equal per-core type counts, padded to tiles ---
    idx0 = np.flatnonzero(t == 0)
    idx1 = np.flatnonzero(t != 0)
    n0, n1 = len(idx0), len(idx1)

    def pad_split(idx, cnt):
        if cnt == 0:
            return np.zeros((CORES, 0), dtype=np.int64), 0
        per = _round_up(-(-cnt // CORES), 16)
        padded = np.concatenate(
            [idx, np.full(CORES * per - cnt, idx[-1], dtype=idx.dtype)]
        )
        return padded.reshape(CORES, per).astype(np.int64), per

    chunks0, P0 = pad_split(idx0, n0)
    chunks1, P1 = pad_split(idx1, n1)

    nc = _build_program(P0, P1)

    # --- weights (shared across cores) ---
    hc2 = h_child.reshape(n, 2 * H)
    cc2 = c_child.reshape(n, 2 * H)

    def bias_tile(v, m):
        # [m*128] bias vector -> [128, m] per-partition layout
        return np.ascontiguousarray(
            np.asarray(v, np.float32).reshape(-1)[: 128 * m].reshape(m, 128).T
        )

    def w_with_bias(W, b):
        # [XPAD, 768] = W^T with the iou bias as row 300, zero-padded to 384
        out = np.zeros((XPAD, 3 * H), dtype=NP_FP16)
        out[:X] = np.asarray(W, np.float32).T.astype(NP_FP16)
        out[X] = np.asarray(b, np.float32).reshape(-1).astype(NP_FP16)
        return out

    wmap = {
        "WnT": w_with_bias(W_iou, b_iou),
        "UnT": np.ascontiguousarray(np.asarray(U_iou, np.float32).T).astype(NP_FP16),
        "UfwT": np.ascontiguousarray(np.asarray(U_f_w, np.float32).T).astype(NP_FP16),
        "WsT": w_with_bias(W_iou_s, b_iou_s),
        "UsT": np.ascontiguousarray(np.asarray(U_iou_s, np.float32).T).astype(NP_FP16),
        "UfswT": np.ascontiguousarray(np.asarray(U_f_s_w, np.float32).T).astype(NP_FP16),
        "bias_fn": bias_tile(U_f_b, 4),
        "bias_fs": bias_tile(U_f_s_b, 2),
    }

    in_maps = []
    for i in range(CORES):
        I = np.concatenate([chunks0[i], chunks1[i]])
        m = dict(wmap)
        xTi = np.zeros((XPAD, len(I)), dtype=NP_FP16)
        xTi[:X] = x[I].T.astype(NP_FP16)
        xTi[X] = 1.0
        m["xT"] = xTi
        m["hT"] = hc2[I].T.astype(NP_FP16)
        m["cT"] = cc2[I].T.astype(NP_FP16)
        in_maps.append(m)

    res = bass_utils.run_bass_kernel_spmd(
        nc, in_maps, core_ids=list(range(CORES)), trace=TRACE
    )
    LAST_EXEC_NS = res.exec_time_ns

    # --- scatter back ---
    h_out = np.empty((n, H), dtype=np.float32)
    c_out = np.empty((n, H), dtype=np.float32)
    if n0:
        h0 = np.concatenate([res.results[i]["hOT"][:, :P0].T for i in range(CORES)])
        c0 = np.concatenate([res.results[i]["cOT"][:, :P0].T for i in range(CORES)])
        h_out[idx0] = h0[:n0].astype(np.float32)
        c_out[idx0] = c0[:n0].astype(np.float32)
    if n1:
        h1 = np.concatenate([res.results[i]["hOT"][:, P0:].T for i in range(CORES)])
        c1 = np.concatenate([res.results[i]["cOT"][:, P0:].T for i in range(CORES)])
        h_out[idx1] = h1[:n1].astype(np.float32)
        c_out[idx1] = c1[:n1].astype(np.float32)
    return h_out, c_out
